# revision 3
# baseline (speedup 1.0000x reference)
import numpy as np
import jax
import jax.numpy as jnp

# Problem constants (hardcoded per spec: nn_DifferentiableEKF)
B, T, M, N, H = 1024, 200, 16, 8, 64
DT, JITTER = 0.01, 1e-6
N_CORES = 8
CHUNK = 25  # time steps per compiled chunk (T % CHUNK == 0)


def _cholesky8(S):
    """Unrolled Cholesky for (Bs, 8, 8) SPD matrices. Returns lower L."""
    cols = [None] * N
    for j in range(N):
        acc = S[:, :, j]  # (Bs, 8) column j
        for k in range(j):
            acc = acc - cols[k] * cols[k][:, j:j + 1]
        inv = 1.0 / jnp.sqrt(acc[:, j:j + 1])
        colj = acc * inv
        mask = jnp.asarray(np.arange(N) >= j, dtype=S.dtype)
        cols[j] = colj * mask
    return jnp.stack(cols, axis=2)  # (Bs, 8, 8)


def _solve_lower(L, Bmat):
    """Solve L @ Y = Bmat, L (Bs,8,8) lower, Bmat (Bs,8,R)."""
    ys = []
    for i in range(N):
        acc = Bmat[:, i, :]
        for k in range(i):
            acc = acc - L[:, i, k:k + 1] * ys[k]
        ys.append(acc / L[:, i, i:i + 1])
    return jnp.stack(ys, axis=1)


def _solve_lower_T(L, Bmat):
    """Solve L.T @ X = Bmat (i.e. trans=1), L lower."""
    xs = [None] * N
    for i in reversed(range(N)):
        acc = Bmat[:, i, :]
        for k in range(i + 1, N):
            acc = acc - L[:, k, i:i + 1] * xs[k]
        xs[i] = acc / L[:, i, i:i + 1]
    return jnp.stack(xs, axis=1)


def _step(x_filt, Sigma_filt, obs_t, W1, b1, W2, b2, C, d, Q, R, eye_m, eye_n):
    h = jnp.tanh(x_filt @ W1 + b1)
    x_pred = x_filt + DT * (h @ W2 + b2)
    F = eye_m + DT * jnp.einsum('ik,bk,kj->bji', W1, 1.0 - h * h, W2)
    Sigma_pred = F @ Sigma_filt @ jnp.swapaxes(F, -1, -2) + Q
    y_pred = x_pred @ C.T + d
    S = jnp.einsum('nm,bmk,lk->bnl', C, Sigma_pred, C) + R
    chol_S = _cholesky8(S + JITTER * eye_n)
    diag = jnp.stack([chol_S[:, i, i] for i in range(N)], axis=1)
    logdet = 2.0 * jnp.sum(jnp.log(diag), axis=-1)
    delta_y = obs_t - y_pred
    Sigma_HT = Sigma_pred @ C.T
    rhs = jnp.swapaxes(Sigma_HT, -1, -2)
    tmp = _solve_lower(chol_S, rhs)
    Kt = _solve_lower_T(chol_S, tmp)
    K = jnp.swapaxes(Kt, -1, -2)
    x_new = x_pred + (K @ delta_y[..., None])[..., 0]
    ImKH = eye_m - K @ C
    Sigma_new = (ImKH @ Sigma_pred @ jnp.swapaxes(ImKH, -1, -2)
                 + K @ R @ jnp.swapaxes(K, -1, -2))
    Sigma_new = 0.5 * (Sigma_new + jnp.swapaxes(Sigma_new, -1, -2))
    whitened = _solve_lower(chol_S, delta_y[..., None])[..., 0]
    outs = (x_pred, x_new, Sigma_pred, Sigma_new, delta_y, S, logdet, whitened)
    return x_new, Sigma_new, outs


def _ekf_chunk(obs_chunk, x, Sigma, W1, b1, W2, b2, C, d, Lq, Lr):
    """Run CHUNK steps, fully unrolled. obs_chunk: (Bs, CHUNK, N)."""
    eye_m = jnp.eye(M, dtype=obs_chunk.dtype)
    eye_n = jnp.eye(N, dtype=obs_chunk.dtype)
    Q = Lq @ Lq.T + 1e-4 * eye_m
    R = Lr @ Lr.T + 1e-4 * eye_n
    per_t = []
    for t in range(CHUNK):
        x, Sigma, outs = _step(x, Sigma, obs_chunk[:, t, :],
                               W1, b1, W2, b2, C, d, Q, R, eye_m, eye_n)
        per_t.append(outs)
    stacked = tuple(jnp.stack([p[i] for p in per_t], axis=1)
                    for i in range(8))
    return (x, Sigma) + stacked


_pmapped_chunk = jax.pmap(
    _ekf_chunk,
    in_axes=(0, 0, 0, None, None, None, None, None, None, None, None),
)


def kernel(**inputs):
    obs = np.asarray(inputs["observations"], dtype=np.float32)
    x0 = np.asarray(inputs["x0"], dtype=np.float32)
    Sigma0 = np.asarray(inputs["Sigma0"], dtype=np.float32)
    Bs = B // N_CORES
    obs_s = obs.reshape(N_CORES, Bs, T, N)
    params = [np.asarray(inputs[k], dtype=np.float32)
              for k in ("W1", "b1", "W2", "b2", "C", "d", "Lq", "Lr")]
    x = jnp.asarray(x0.reshape(N_CORES, Bs, M))
    Sigma = jnp.asarray(Sigma0.reshape(N_CORES, Bs, M, M))
    chunk_outs = []
    for c in range(T // CHUNK):
        res = _pmapped_chunk(
            obs_s[:, :, c * CHUNK:(c + 1) * CHUNK, :], x, Sigma, *params)
        x, Sigma = res[0], res[1]
        chunk_outs.append(res[2:])
    full = []
    for i in range(8):
        o = np.concatenate([np.asarray(c[i]) for c in chunk_outs], axis=2)
        full.append(o.reshape((B,) + o.shape[2:]))
    return tuple(full)


# revision 4
# speedup vs baseline: 1.0465x; 1.0465x over previous
import numpy as np
import jax
import jax.numpy as jnp

# Problem constants (hardcoded per spec: nn_DifferentiableEKF)
B, T, M, N, H = 1024, 200, 16, 8, 64
DT, JITTER = 0.01, 1e-6
N_CORES = 8
CHUNK = 25  # time steps per compiled chunk (T % CHUNK == 0)


def _cholesky8(S):
    """Unrolled Cholesky for (Bs, 8, 8) SPD matrices. Returns lower L."""
    cols = [None] * N
    for j in range(N):
        acc = S[:, :, j]  # (Bs, 8) column j
        for k in range(j):
            acc = acc - cols[k] * cols[k][:, j:j + 1]
        inv = 1.0 / jnp.sqrt(acc[:, j:j + 1])
        colj = acc * inv
        mask = jnp.asarray(np.arange(N) >= j, dtype=S.dtype)
        cols[j] = colj * mask
    return jnp.stack(cols, axis=2)  # (Bs, 8, 8)


def _solve_lower(L, Bmat):
    """Solve L @ Y = Bmat, L (Bs,8,8) lower, Bmat (Bs,8,R)."""
    ys = []
    for i in range(N):
        acc = Bmat[:, i, :]
        for k in range(i):
            acc = acc - L[:, i, k:k + 1] * ys[k]
        ys.append(acc / L[:, i, i:i + 1])
    return jnp.stack(ys, axis=1)


def _solve_lower_T(L, Bmat):
    """Solve L.T @ X = Bmat (i.e. trans=1), L lower."""
    xs = [None] * N
    for i in reversed(range(N)):
        acc = Bmat[:, i, :]
        for k in range(i + 1, N):
            acc = acc - L[:, k, i:i + 1] * xs[k]
        xs[i] = acc / L[:, i, i:i + 1]
    return jnp.stack(xs, axis=1)


def _step(x_filt, Sigma_filt, obs_t, W1, b1, W2, b2, C, d, Q, R, eye_m, eye_n):
    h = jnp.tanh(x_filt @ W1 + b1)
    x_pred = x_filt + DT * (h @ W2 + b2)
    s = 1.0 - h * h  # (Bs, H)
    # F = I + DT * W2.T diag(s) W1.T (per batch). Never materialize F:
    # with Sigma symmetric, Y^T = DT*((Sigma@W1)*s)@W2 where Y = (F-I)Sigma.
    P = (Sigma_filt @ W1) * s[:, None, :]          # (Bs,16,64)
    YT = DT * (P @ W2)                              # YT[b,j,m] = Y[b,m,j]
    Y = jnp.swapaxes(YT, 1, 2)
    # Z2 = (F-I) Y^T = (F-I) Sigma (F-I)^T (symmetric)
    P2 = (Y @ W1) * s[:, None, :]
    Z2 = DT * (P2 @ W2)
    Sigma_pred = Sigma_filt + Y + YT + Z2 + Q
    y_pred = x_pred @ C.T + d
    SH = Sigma_pred @ C.T                           # (Bs,16,8)
    rhs = jnp.swapaxes(SH, 1, 2)                    # = C @ Sigma_pred (sym)
    S = rhs @ C.T + R                               # (Bs,8,8)
    chol_S = _cholesky8(S + JITTER * eye_n)
    diag = jnp.stack([chol_S[:, i, i] for i in range(N)], axis=1)
    logdet = 2.0 * jnp.sum(jnp.log(diag), axis=-1)
    delta_y = obs_t - y_pred
    # one forward solve with 17 stacked RHS: [C Sigma_pred | delta_y]
    rhs17 = jnp.concatenate([rhs, delta_y[..., None]], axis=2)
    tmp17 = _solve_lower(chol_S, rhs17)             # (Bs,8,17)
    tmp = tmp17[:, :, :M]                           # L^-1 C Sigma_pred
    whitened = tmp17[:, :, M]                       # L^-1 delta_y
    # K delta_y = tmp^T whitened;  Joseph form reduces exactly to
    # Sigma_pred - tmp^T tmp  (= Sigma - K C Sigma for K = Sigma C^T S^-1).
    x_new = x_pred + jnp.einsum('brm,br->bm', tmp, whitened)
    Sigma_new = Sigma_pred - jnp.einsum('brm,brk->bmk', tmp, tmp)
    Sigma_new = 0.5 * (Sigma_new + jnp.swapaxes(Sigma_new, -1, -2))
    outs = (x_pred, x_new, Sigma_pred, Sigma_new, delta_y, S, logdet, whitened)
    return x_new, Sigma_new, outs


def _ekf_chunk(obs_chunk, x, Sigma, W1, b1, W2, b2, C, d, Lq, Lr):
    """Run CHUNK steps, fully unrolled. obs_chunk: (Bs, CHUNK, N)."""
    eye_m = jnp.eye(M, dtype=obs_chunk.dtype)
    eye_n = jnp.eye(N, dtype=obs_chunk.dtype)
    Q = Lq @ Lq.T + 1e-4 * eye_m
    R = Lr @ Lr.T + 1e-4 * eye_n
    per_t = []
    for t in range(CHUNK):
        x, Sigma, outs = _step(x, Sigma, obs_chunk[:, t, :],
                               W1, b1, W2, b2, C, d, Q, R, eye_m, eye_n)
        per_t.append(outs)
    stacked = tuple(jnp.stack([p[i] for p in per_t], axis=1)
                    for i in range(8))
    return (x, Sigma) + stacked


_pmapped_chunk = jax.pmap(
    _ekf_chunk,
    in_axes=(0, 0, 0, None, None, None, None, None, None, None, None),
)


def kernel(**inputs):
    obs = np.asarray(inputs["observations"], dtype=np.float32)
    x0 = np.asarray(inputs["x0"], dtype=np.float32)
    Sigma0 = np.asarray(inputs["Sigma0"], dtype=np.float32)
    Bs = B // N_CORES
    obs_s = obs.reshape(N_CORES, Bs, T, N)
    params = [np.asarray(inputs[k], dtype=np.float32)
              for k in ("W1", "b1", "W2", "b2", "C", "d", "Lq", "Lr")]
    x = jnp.asarray(x0.reshape(N_CORES, Bs, M))
    Sigma = jnp.asarray(Sigma0.reshape(N_CORES, Bs, M, M))
    chunk_outs = []
    for c in range(T // CHUNK):
        res = _pmapped_chunk(
            obs_s[:, :, c * CHUNK:(c + 1) * CHUNK, :], x, Sigma, *params)
        x, Sigma = res[0], res[1]
        chunk_outs.append(res[2:])
    full = []
    for i in range(8):
        o = np.concatenate([np.asarray(c[i]) for c in chunk_outs], axis=2)
        full.append(o.reshape((B,) + o.shape[2:]))
    return tuple(full)
